# revision 1
# baseline (speedup 1.0000x reference)
"""Trainium2 Bass kernel for nn_Decoder_46660524704357.

Reference computation (shapes hardcoded in DEFAULT_CFG):
    B, C, L, D, E, K = 64, 23, 26000, 64, 512, 3
    eos  = eos_emb @ eos_W.T + eos_b          # [B,C,D]
    bin_emb = emb_table[bin_ids]              # [C,L,D]
    a = bin_emb @ Wb.T                        # [C,L,K]   Wb = fc_W[:, :D]
    e = eos @ We.T + fc_b                     # [B,C,K]   We = fc_W[:, D:]
    out = relu(a[None,:,:,:] + e[:,:,None,:]) # [B,C,L,K]

Sharding: split L across the 8 cores (Lc = 3250 each).  Each core:
  - computes the eos projection chain (tiny) to get e[B,C,K] on-device,
  - for each chromosome c and each output tile, runs ONE fused matmul:
        out[p=(k*B+b), l] = sum_d Wsel[d, p] * embT[d, l] + e_row[p] * 1
    where Wsel[d, k*B+b] = fc_W[k, d] (constant) and the (D+1)-th contract
    row of embT is all-ones so the e term rides along.  ScalarE applies ReLU
    on the PSUM->SBUF copy, DMA writes a [B, C, K, Lc] layout output.
Host re-interleaves K innermost at the end.
"""

import numpy as np

DEFAULT_CFG = dict(B=64, C=23, L=26000, D=64, E=512, K=3, NCORES=8)

_CACHE = {}


def _derived(cfg):
    B, C, L, D, E, K, NCORES = (cfg[k] for k in ("B", "C", "L", "D", "E", "K", "NCORES"))
    d = dict(cfg)
    d["LC"] = L // NCORES
    d["BC"] = B * C
    d["EP"] = min(128, E)              # contract chunk for eos matmul
    assert E % d["EP"] == 0
    d["NQ"] = E // d["EP"]
    d["ROWS"] = K * B                  # output partition rows (b*K + k)
    # partition tiles over ROWS: cut at b boundaries so each tile's DMA rows
    # merge into contiguous [K*LC] runs per b
    tiles = []
    bmax = 128 // K                    # b's per tile
    b0 = 0
    while b0 < B:
        nb = min(bmax, B - b0)
        tiles.append((b0 * K, nb * K, b0, nb))
        b0 += nb
    d["PTILES"] = tiles                # (p_off, p_n, b0, nb)
    fc = min(512, d["LC"])
    d["NF"] = [fc] * (d["LC"] // fc) + ([d["LC"] % fc] if d["LC"] % fc else [])
    return d


def _build_nc(cfg=None, selector_f32r=True, eos_f32r=False):
    import concourse.bass as bass  # noqa: F401
    import concourse.mybir as mybir
    import concourse.tile as tile
    from concourse import bacc

    g = _derived(cfg or DEFAULT_CFG)
    B, C, D, K = g["B"], g["C"], g["D"], g["K"]
    LC, BC, EP, NQ, ROWS = g["LC"], g["BC"], g["EP"], g["NQ"], g["ROWS"]
    FCH = min(512, BC)

    f32 = mybir.dt.float32
    f32r = mybir.dt.float32r
    # dtype for tensors consumed by the big selector matmul: fp32r streams
    # 1 col/cycle (vs 4 for fp32).  The BIR verifier requires the whole
    # producer chain to carry the f32r dtype.
    fsel = f32r if selector_f32r else f32
    feos = f32r if eos_f32r else f32

    # Bacc (not plain Bass): its compile() passes split multi-sem waits and
    # move matmul waits to ldweights — required for walrus codegen.
    nc = bacc.Bacc(None)

    embT = nc.declare_dram_parameter("embT", [D + 1, C * LC], fsel, isOutput=False)
    eosE = nc.declare_dram_parameter("eosE", [EP, NQ * BC], feos, isOutput=False)
    eosW = nc.declare_dram_parameter("eosW", [EP, NQ * D], feos, isOutput=False)
    WeT = nc.declare_dram_parameter("WeT", [D, K], feos, isOutput=False)
    eos_b = nc.declare_dram_parameter("eos_b", [D, 1], f32, isOutput=False)
    fc_b = nc.declare_dram_parameter("fc_b", [1, K], f32, isOutput=False)
    wsel = nc.declare_dram_parameter("wsel", [D, C * ROWS], fsel, isOutput=False)
    out = nc.declare_dram_parameter("out", [B, C, K, LC], f32, isOutput=True)

    with tile.TileContext(nc) as tc:
        with (
            tc.tile_pool(name="consts", bufs=1) as consts,
            tc.tile_pool(name="setup_sb", bufs=1) as setup_sb,
            tc.tile_pool(name="setup_ps", bufs=1, space="PSUM") as setup_ps,
            tc.tile_pool(name="emb", bufs=3) as emb_pool,
            tc.tile_pool(name="osb", bufs=5) as osb_pool,
            tc.tile_pool(name="ops", bufs=6, space="PSUM") as ops_pool,
        ):
            # ---- constants / setup -------------------------------------
            # all setup loads ride gpsimd's SWDGE (16-way engine fan-out);
            # eosE is chunked so the eos matmuls start on the first chunk
            se = consts.tile([D + 1, C * ROWS], fsel)        # selector weights
            nc.gpsimd.dma_start(se[0:D, :], wsel[:, :])

            eosE_sb = setup_sb.tile([EP, NQ * BC], feos)
            for q in range(NQ):
                nc.gpsimd.dma_start(
                    eosE_sb[:, q * BC:(q + 1) * BC],
                    eosE[:, q * BC:(q + 1) * BC])
            eosW_sb = setup_sb.tile([EP, NQ * D], feos)
            nc.gpsimd.dma_start(eosW_sb[:, :], eosW[:, :])
            WeT_sb = setup_sb.tile([D, K], f32)
            nc.sync.dma_start(WeT_sb[:, :], WeT[:, :])
            eosb_sb = setup_sb.tile([D, 1], f32)
            nc.sync.dma_start(eosb_sb[:, :], eos_b[:, :])
            fcb_sb = setup_sb.tile([1, K], f32)
            nc.sync.dma_start(fcb_sb[:, :], fc_b[:, :])

            # eosT[d, bc] = sum_E eos_W[d, E] * eos_emb[bc, E]  (+ eos_b)
            eosT_sb = setup_sb.tile([D, BC], feos)
            bc_chunks = [(i, min(FCH, BC - i)) for i in range(0, BC, FCH)]
            for bc0, nbc in bc_chunks:
                eosT_ps = setup_ps.tile([D, nbc], f32, tag="eos_ps")
                for q in range(NQ):
                    nc.tensor.matmul(
                        eosT_ps[:, :],
                        lhsT=eosW_sb[:, q * D:(q + 1) * D],
                        rhs=eosE_sb[:, q * BC + bc0: q * BC + bc0 + nbc],
                        start=(q == 0),
                        stop=(q == NQ - 1),
                    )
                nc.scalar.add(eosT_sb[:, bc0:bc0 + nbc], eosT_ps[:, :], eosb_sb[:, 0:1])

            # e_row[p=(c,b,k)] = sum_d We[k,d]*eosT[d,bc] + fc_b[k], computed
            # directly in selector-column order:
            #   X[d, (c,b,k)] = eosT[d, b*C+c] * WeT[d, k]   (DVE, bcast APs)
            #   X[0, :]      += fc_b[k]                       (DVE)
            #   e_row         = colsum(X)                     (PE, ones lhsT)
            X = setup_sb.tile([D, C * ROWS], f32)
            eosT_g = eosT_sb[:, :].rearrange("d (b c) -> d c b", b=B, c=C) \
                .unsqueeze(3).broadcast_to([D, C, B, K])
            We_g = WeT_sb[:, :].unsqueeze(1).unsqueeze(1).broadcast_to([D, C, B, K])
            X_w = X[:, :].rearrange("d (c b k) -> d c b k", c=C, b=B, k=K)
            nc.vector.tensor_mul(X_w, eosT_g, We_g)
            fcb_g = fcb_sb[:, :].unsqueeze(1).unsqueeze(1).broadcast_to([1, C, B, K])
            X0_w = X[0:1, :].rearrange("d (c b k) -> d c b k", c=C, b=B, k=K)
            nc.vector.tensor_add(X0_w, X0_w.copy(), fcb_g)

            ones64 = setup_sb.tile([D, 1], f32)
            nc.vector.memset(ones64[:, :], 1.0)
            row_chunks = [(i, min(512, C * ROWS - i)) for i in range(0, C * ROWS, 512)]
            for r0, nr in row_chunks:
                e_ps = setup_ps.tile([D + 1, nr], f32, tag="eos_ps")
                nc.tensor.matmul(
                    e_ps[D:D + 1, :],
                    lhsT=ones64[:, 0:1],
                    rhs=X[:, r0:r0 + nr],
                    start=True,
                    stop=True,
                )
                nc.scalar.activation(
                    se[D:D + 1, r0:r0 + nr], e_ps[D:D + 1, :],
                    mybir.ActivationFunctionType.Copy,
                )

            # ---- main loop ---------------------------------------------
            # DMA engine assignment: each engine's dynamic-HW queue is a
            # separate hardware ring, and each dma_start trigger costs ~1us
            # on the issuing engine — so spread big transfers across
            # engines and keep the count low (one out-DMA per (c, ptile)).
            out_bkl = out.rearrange("b c k l -> c b (k l)")
            # weighted round-robin: gpsimd's software DGE spreads descriptors
            # across all 16 DMA engines; the sync/scalar hardware queues only
            # reach 2-3 engines each
            out_engines = [nc.gpsimd, nc.gpsimd, nc.sync, nc.gpsimd, nc.gpsimd, nc.scalar]
            n_dma = 0
            for c in range(C):
                # et via gpsimd SWDGE: one DMA's descriptors fan out across
                # all 16 DMA engines (HW queues only reach 2-3), so the load
                # latency is ~16x lower — keeps PE fed
                et = emb_pool.tile([D + 1, LC], fsel, tag="embT")
                nc.gpsimd.dma_start(et[:, :], embT[:, c * LC:(c + 1) * LC])
                for ti, (p_off, p_n, b0, nb) in enumerate(g["PTILES"]):
                    so = osb_pool.tile([p_n, LC], f32, tag="out_sb")
                    f0 = 0
                    for nf in g["NF"]:
                        po = ops_pool.tile([p_n, nf], f32, tag="out_ps")
                        nc.tensor.matmul(
                            po[:, :],
                            lhsT=se[:, c * ROWS + p_off: c * ROWS + p_off + p_n],
                            rhs=et[:, f0:f0 + nf],
                            start=True,
                            stop=True,
                        )
                        # ReLU on PSUM->SBUF copy; alternate engines so the
                        # scalar engine isn't the serial resource
                        if ti % 2 == 0:
                            nc.scalar.activation(
                                so[:, f0:f0 + nf], po[:, :],
                                mybir.ActivationFunctionType.Relu,
                            )
                        else:
                            nc.vector.tensor_scalar_max(so[:, f0:f0 + nf], po[:, :], 0.0)
                        f0 += nf
                    out_engines[n_dma % len(out_engines)].dma_start(
                        out_bkl[c, b0:b0 + nb, :], so[:, :]
                    )
                    n_dma += 1
    nc.finalize()
    return nc


def _host_prep(eos_emb, bin_ids, emb_table, eos_W, eos_b, fc_W, fc_b, cfg=None):
    """Build the per-core input maps."""
    g = _derived(cfg or DEFAULT_CFG)
    B, C, L, D, E, K = g["B"], g["C"], g["L"], g["D"], g["E"], g["K"]
    NCORES, LC, BC, EP, NQ, ROWS = (
        g["NCORES"], g["LC"], g["BC"], g["EP"], g["NQ"], g["ROWS"])

    eos_emb = np.ascontiguousarray(eos_emb, dtype=np.float32)
    emb_table = np.ascontiguousarray(emb_table, dtype=np.float32)
    bin_ids = np.asarray(bin_ids)

    # gather (identity when bin_ids == arange, which is the spec'd fill)
    V = C * L
    flat_ids = bin_ids.reshape(-1)
    if flat_ids.shape[0] == V and emb_table.shape[0] == V and \
            flat_ids[0] == 0 and flat_ids[-1] == V - 1 and \
            np.array_equal(flat_ids, np.arange(V, dtype=flat_ids.dtype)):
        bin_emb = emb_table.reshape(C, L, D)
    else:
        bin_emb = emb_table[bin_ids.reshape(C, L)]

    eosE = np.ascontiguousarray(
        eos_emb.reshape(BC, E).T.reshape(NQ, EP, BC).transpose(1, 0, 2).reshape(EP, NQ * BC)
    )
    eosW = np.ascontiguousarray(
        np.asarray(eos_W, np.float32).T.reshape(NQ, EP, D).transpose(1, 0, 2).reshape(EP, NQ * D)
    )
    fc_W = np.asarray(fc_W, np.float32)
    WeT = np.ascontiguousarray(fc_W[:, D:].T)            # [D, K]
    eos_b_in = np.asarray(eos_b, np.float32).reshape(D, 1)
    fc_b_in = np.asarray(fc_b, np.float32).reshape(1, K)
    # wsel[d, c*ROWS + b*K + k] = fc_W[k, d]
    wsel1 = np.tile(fc_W[:, :D], (B, 1)).T               # [D, B*K] (b-major)
    wsel = np.ascontiguousarray(np.tile(wsel1, (1, C)))  # [D, C*ROWS]

    shared = dict(eosE=eosE, eosW=eosW, WeT=WeT, eos_b=eos_b_in, fc_b=fc_b_in, wsel=wsel)

    in_maps = []
    for i in range(NCORES):
        sl = bin_emb[:, i * LC:(i + 1) * LC, :]          # [C, Lc, D]
        embT_i = np.empty((D + 1, C * LC), np.float32)
        embT_i[:D] = sl.transpose(2, 0, 1).reshape(D, C * LC)
        embT_i[D] = 1.0
        in_maps.append({"embT": embT_i, **shared})
    return in_maps


def _assemble(results, cfg=None):
    g = _derived(cfg or DEFAULT_CFG)
    B, C, L, K, NCORES, LC = g["B"], g["C"], g["L"], g["K"], g["NCORES"], g["LC"]
    out = np.empty((B, C, L, K), np.float32)
    for i in range(NCORES):
        r = results[i]["out"]                            # [B, C, K, Lc]
        out[:, :, i * LC:(i + 1) * LC, :] = r.transpose(0, 1, 3, 2)
    return out


def kernel(eos_emb, bin_ids, emb_table, eos_W, eos_b, fc_W, fc_b):
    from concourse.bass_utils import run_bass_kernel_spmd

    if "nc" not in _CACHE:
        _CACHE["nc"] = _build_nc()
    nc = _CACHE["nc"]
    in_maps = _host_prep(eos_emb, bin_ids, emb_table, eos_W, eos_b, fc_W, fc_b)
    res = run_bass_kernel_spmd(nc, in_maps, core_ids=list(range(DEFAULT_CFG["NCORES"])))
    return _assemble(res.results)



# revision 2
# speedup vs baseline: 1.6451x; 1.6451x over previous
"""Trainium2 Bass kernel for nn_Decoder_46660524704357.

Reference computation (shapes hardcoded in DEFAULT_CFG):
    B, C, L, D, E, K = 64, 23, 26000, 64, 512, 3
    eos  = eos_emb @ eos_W.T + eos_b          # [B,C,D]
    bin_emb = emb_table[bin_ids]              # [C,L,D]
    a = bin_emb @ Wb.T                        # [C,L,K]   Wb = fc_W[:, :D]
    e = eos @ We.T + fc_b                     # [B,C,K]   We = fc_W[:, D:]
    out = relu(a[None,:,:,:] + e[:,:,None,:]) # [B,C,L,K]

Sharding: split L across the 8 cores (Lc = 3250 each).  Each core:
  - computes the eos projection chain (tiny) to get e[B,C,K] on-device,
  - for each chromosome c and each output tile, runs ONE fused matmul:
        out[p=(b*K+k), l] = sum_d Wsel[d, p] * embT[d, l] + e_row[p] * 1
    where Wsel[d, b*K+k] = fc_W[k, d] (constant) and the (D+1)-th contract
    row of embT is all-ones so the e term rides along.  ScalarE/VectorE
    apply ReLU on the PSUM->SBUF copy, DMA writes a [B, C, K, Lc] output.

v2: all-bf16 datapath (embT / selector / eos chain / output staging and
DRAM output) -- halves HBM traffic both ways; accumulation stays fp32 in
PSUM.  The 2e-2 rel-err budget (~0.063 absolute at |out|max~3.1) dwarfs
bf16 rounding (~0.01).  DMA queue separation: embT loads ride the sync
HWDGE ring so they are never stuck behind output stores; stores alternate
between the gpsimd SWDGE ring and the scalar HWDGE ring.  Host upcasts
the bf16 output to fp32 and re-interleaves K innermost.
"""

import numpy as np

DEFAULT_CFG = dict(B=64, C=23, L=26000, D=64, E=512, K=3, NCORES=8)

_CACHE = {}


def _derived(cfg):
    B, C, L, D, E, K, NCORES = (cfg[k] for k in ("B", "C", "L", "D", "E", "K", "NCORES"))
    d = dict(cfg)
    d["LC"] = L // NCORES
    d["BC"] = B * C
    d["EP"] = min(128, E)              # contract chunk for eos matmul
    assert E % d["EP"] == 0
    d["NQ"] = E // d["EP"]
    d["ROWS"] = K * B                  # output partition rows (b*K + k)
    # partition tiles over ROWS: cut at b boundaries so each tile's DMA rows
    # merge into contiguous [K*LC] runs per b
    tiles = []
    bmax = 128 // K                    # b's per tile
    b0 = 0
    while b0 < B:
        nb = min(bmax, B - b0)
        tiles.append((b0 * K, nb * K, b0, nb))
        b0 += nb
    d["PTILES"] = tiles                # (p_off, p_n, b0, nb)
    fc = min(512, d["LC"])
    d["NF"] = [fc] * (d["LC"] // fc) + ([d["LC"] % fc] if d["LC"] % fc else [])
    return d


def _build_nc(cfg=None):
    import concourse.bass as bass  # noqa: F401
    import concourse.mybir as mybir
    import concourse.tile as tile
    from concourse import bacc

    g = _derived(cfg or DEFAULT_CFG)
    B, C, D, K = g["B"], g["C"], g["D"], g["K"]
    LC, BC, EP, NQ, ROWS = g["LC"], g["BC"], g["EP"], g["NQ"], g["ROWS"]
    FCH = min(512, BC)

    f32 = mybir.dt.float32
    bf16 = mybir.dt.bfloat16
    fsel = bf16                         # selector matmul operand dtype
    feos = bf16                         # eos matmul operand dtype

    # Bacc (not plain Bass): its compile() passes split multi-sem waits and
    # move matmul waits to ldweights — required for walrus codegen.
    nc = bacc.Bacc(None)

    embT = nc.declare_dram_parameter("embT", [D + 1, C * LC], fsel, isOutput=False)
    eosE = nc.declare_dram_parameter("eosE", [EP, NQ * BC], feos, isOutput=False)
    eosW = nc.declare_dram_parameter("eosW", [EP, NQ * D], feos, isOutput=False)
    WeT = nc.declare_dram_parameter("WeT", [D, K], bf16, isOutput=False)
    eos_b = nc.declare_dram_parameter("eos_b", [D, 1], f32, isOutput=False)
    fc_b = nc.declare_dram_parameter("fc_b", [1, K], bf16, isOutput=False)
    wsel = nc.declare_dram_parameter("wsel", [D, C * ROWS], fsel, isOutput=False)
    out = nc.declare_dram_parameter("out", [B, C, K, LC], bf16, isOutput=True)

    with tile.TileContext(nc) as tc:
        with (
            tc.tile_pool(name="consts", bufs=1) as consts,
            tc.tile_pool(name="setup_sb", bufs=1) as setup_sb,
            tc.tile_pool(name="setup_ps", bufs=1, space="PSUM") as setup_ps,
            tc.tile_pool(name="emb", bufs=4) as emb_pool,
            tc.tile_pool(name="osb", bufs=8) as osb_pool,
            tc.tile_pool(name="ops", bufs=6, space="PSUM") as ops_pool,
        ):
            # ---- constants / setup -------------------------------------
            se = consts.tile([D + 1, C * ROWS], fsel)        # selector weights
            nc.gpsimd.dma_start(se[0:D, :], wsel[:, :])

            eosE_sb = setup_sb.tile([EP, NQ * BC], feos)
            for q in range(NQ):
                nc.gpsimd.dma_start(
                    eosE_sb[:, q * BC:(q + 1) * BC],
                    eosE[:, q * BC:(q + 1) * BC])
            eosW_sb = setup_sb.tile([EP, NQ * D], feos)
            nc.gpsimd.dma_start(eosW_sb[:, :], eosW[:, :])
            WeT_sb = setup_sb.tile([D, K], bf16)
            nc.sync.dma_start(WeT_sb[:, :], WeT[:, :])
            eosb_sb = setup_sb.tile([D, 1], f32)
            nc.sync.dma_start(eosb_sb[:, :], eos_b[:, :])
            fcb_sb = setup_sb.tile([1, K], bf16)
            nc.sync.dma_start(fcb_sb[:, :], fc_b[:, :])

            # eosT[d, bc] = sum_E eos_W[d, E] * eos_emb[bc, E]  (+ eos_b)
            eosT_sb = setup_sb.tile([D, BC], bf16)
            bc_chunks = [(i, min(FCH, BC - i)) for i in range(0, BC, FCH)]
            for bc0, nbc in bc_chunks:
                eosT_ps = setup_ps.tile([D, nbc], f32, tag="eos_ps")
                for q in range(NQ):
                    nc.tensor.matmul(
                        eosT_ps[:, :],
                        lhsT=eosW_sb[:, q * D:(q + 1) * D],
                        rhs=eosE_sb[:, q * BC + bc0: q * BC + bc0 + nbc],
                        start=(q == 0),
                        stop=(q == NQ - 1),
                    )
                nc.scalar.add(eosT_sb[:, bc0:bc0 + nbc], eosT_ps[:, :], eosb_sb[:, 0:1])

            # e_row[p=(c,b,k)] = sum_d We[k,d]*eosT[d,bc] + fc_b[k], computed
            # directly in selector-column order:
            #   X[d, (c,b,k)] = eosT[d, b*C+c] * WeT[d, k]   (DVE, bcast APs)
            #   X[0, :]      += fc_b[k]                       (DVE)
            #   e_row         = colsum(X)                     (PE, ones lhsT)
            X = setup_sb.tile([D, C * ROWS], bf16)
            eosT_g = eosT_sb[:, :].rearrange("d (b c) -> d c b", b=B, c=C) \
                .unsqueeze(3).broadcast_to([D, C, B, K])
            We_g = WeT_sb[:, :].unsqueeze(1).unsqueeze(1).broadcast_to([D, C, B, K])
            X_w = X[:, :].rearrange("d (c b k) -> d c b k", c=C, b=B, k=K)
            nc.vector.tensor_mul(X_w, eosT_g, We_g)
            fcb_g = fcb_sb[:, :].unsqueeze(1).unsqueeze(1).broadcast_to([1, C, B, K])
            X0_w = X[0:1, :].rearrange("d (c b k) -> d c b k", c=C, b=B, k=K)
            nc.vector.tensor_add(X0_w, X0_w.copy(), fcb_g)

            ones64 = setup_sb.tile([D, 1], bf16)
            nc.vector.memset(ones64[:, :], 1.0)
            row_chunks = [(i, min(512, C * ROWS - i)) for i in range(0, C * ROWS, 512)]
            for r0, nr in row_chunks:
                e_ps = setup_ps.tile([D + 1, nr], f32, tag="eos_ps")
                nc.tensor.matmul(
                    e_ps[D:D + 1, :],
                    lhsT=ones64[:, 0:1],
                    rhs=X[:, r0:r0 + nr],
                    start=True,
                    stop=True,
                )
                nc.scalar.activation(
                    se[D:D + 1, r0:r0 + nr], e_ps[D:D + 1, :],
                    mybir.ActivationFunctionType.Copy,
                )

            # ---- main loop ---------------------------------------------
            # DMA ring assignment: embT loads ride the sync HWDGE ring so
            # they are never queued behind output stores; stores alternate
            # gpsimd (SWDGE, 16-engine fan-out) and scalar (HWDGE) rings.
            # Each SDMA engine round-robins across rings at packet
            # granularity, so loads always make progress.
            out_bkl = out.rearrange("b c k l -> c b (k l)")
            store_engines = [nc.gpsimd, nc.scalar]
            n_dma = 0
            for c in range(C):
                et = emb_pool.tile([D + 1, LC], fsel, tag="embT")
                nc.sync.dma_start(et[:, :], embT[:, c * LC:(c + 1) * LC])
                for ti, (p_off, p_n, b0, nb) in enumerate(g["PTILES"]):
                    so = osb_pool.tile([p_n, LC], bf16, tag="out_sb")
                    f0 = 0
                    for fi, nf in enumerate(g["NF"]):
                        po = ops_pool.tile([p_n, nf], f32, tag="out_ps")
                        nc.tensor.matmul(
                            po[:, :],
                            lhsT=se[:, c * ROWS + p_off: c * ROWS + p_off + p_n],
                            rhs=et[:, f0:f0 + nf],
                            start=True,
                            stop=True,
                        )
                        # ReLU on the PSUM->SBUF copy; alternate engines so
                        # neither scalar nor vector is the serial resource
                        if (ti + fi) % 2 == 0:
                            nc.scalar.activation(
                                so[:, f0:f0 + nf], po[:, :],
                                mybir.ActivationFunctionType.Relu,
                            )
                        else:
                            nc.vector.tensor_scalar_max(so[:, f0:f0 + nf], po[:, :], 0.0)
                        f0 += nf
                    store_engines[n_dma % len(store_engines)].dma_start(
                        out_bkl[c, b0:b0 + nb, :], so[:, :]
                    )
                    n_dma += 1
    nc.finalize()
    return nc


def _host_prep(eos_emb, bin_ids, emb_table, eos_W, eos_b, fc_W, fc_b, cfg=None):
    """Build the per-core input maps."""
    import ml_dtypes

    bf16 = ml_dtypes.bfloat16
    g = _derived(cfg or DEFAULT_CFG)
    B, C, L, D, E, K = g["B"], g["C"], g["L"], g["D"], g["E"], g["K"]
    NCORES, LC, BC, EP, NQ, ROWS = (
        g["NCORES"], g["LC"], g["BC"], g["EP"], g["NQ"], g["ROWS"])

    eos_emb = np.ascontiguousarray(eos_emb, dtype=np.float32)
    emb_table = np.ascontiguousarray(emb_table, dtype=np.float32)
    bin_ids = np.asarray(bin_ids)

    # gather (identity when bin_ids == arange, which is the spec'd fill)
    V = C * L
    flat_ids = bin_ids.reshape(-1)
    if flat_ids.shape[0] == V and emb_table.shape[0] == V and \
            flat_ids[0] == 0 and flat_ids[-1] == V - 1 and \
            np.array_equal(flat_ids, np.arange(V, dtype=flat_ids.dtype)):
        bin_emb = emb_table.reshape(C, L, D)
    else:
        bin_emb = emb_table[bin_ids.reshape(C, L)]

    eosE = np.ascontiguousarray(
        eos_emb.reshape(BC, E).T.reshape(NQ, EP, BC).transpose(1, 0, 2).reshape(EP, NQ * BC)
    ).astype(bf16)
    eosW = np.ascontiguousarray(
        np.asarray(eos_W, np.float32).T.reshape(NQ, EP, D).transpose(1, 0, 2).reshape(EP, NQ * D)
    ).astype(bf16)
    fc_W = np.asarray(fc_W, np.float32)
    WeT = np.ascontiguousarray(fc_W[:, D:].T).astype(bf16)   # [D, K]
    eos_b_in = np.asarray(eos_b, np.float32).reshape(D, 1)
    fc_b_in = np.asarray(fc_b, np.float32).reshape(1, K).astype(bf16)
    # wsel[d, c*ROWS + b*K + k] = fc_W[k, d]
    wsel1 = np.tile(fc_W[:, :D], (B, 1)).T               # [D, B*K] (b-major)
    wsel = np.ascontiguousarray(np.tile(wsel1, (1, C))).astype(bf16)

    shared = dict(eosE=eosE, eosW=eosW, WeT=WeT, eos_b=eos_b_in, fc_b=fc_b_in, wsel=wsel)

    in_maps = []
    for i in range(NCORES):
        sl = bin_emb[:, i * LC:(i + 1) * LC, :]          # [C, Lc, D]
        embT_i = np.empty((D + 1, C * LC), bf16)
        embT_i[:D] = sl.transpose(2, 0, 1).reshape(D, C * LC).astype(bf16)
        embT_i[D] = 1.0
        in_maps.append({"embT": embT_i, **shared})
    return in_maps


def _assemble(results, cfg=None):
    g = _derived(cfg or DEFAULT_CFG)
    B, C, L, K, NCORES, LC = g["B"], g["C"], g["L"], g["K"], g["NCORES"], g["LC"]
    out = np.empty((B, C, L, K), np.float32)
    for i in range(NCORES):
        r = results[i]["out"]                            # [B, C, K, Lc] bf16
        out[:, :, i * LC:(i + 1) * LC, :] = r.transpose(0, 1, 3, 2).astype(np.float32)
    return out


def kernel(eos_emb, bin_ids, emb_table, eos_W, eos_b, fc_W, fc_b):
    from concourse.bass_utils import run_bass_kernel_spmd

    if "nc" not in _CACHE:
        _CACHE["nc"] = _build_nc()
    nc = _CACHE["nc"]
    in_maps = _host_prep(eos_emb, bin_ids, emb_table, eos_W, eos_b, fc_W, fc_b)
    res = run_bass_kernel_spmd(nc, in_maps, core_ids=list(range(DEFAULT_CFG["NCORES"])))
    return _assemble(res.results)


# revision 5
# speedup vs baseline: 1.7511x; 1.0645x over previous
"""Trainium2 Bass kernel for nn_Decoder_46660524704357.

Reference computation (shapes hardcoded in DEFAULT_CFG):
    B, C, L, D, E, K = 64, 23, 26000, 64, 512, 3
    eos  = eos_emb @ eos_W.T + eos_b          # [B,C,D]
    bin_emb = emb_table[bin_ids]              # [C,L,D]
    a = bin_emb @ Wb.T                        # [C,L,K]   Wb = fc_W[:, :D]
    e = eos @ We.T + fc_b                     # [B,C,K]   We = fc_W[:, D:]
    out = relu(a[None,:,:,:] + e[:,:,None,:]) # [B,C,L,K]

Sharding: split L across the 8 cores (Lc = 3250 each).  Each core:
  - computes e[B,C,K] on-device via two small matmul chains,
  - for each chromosome c and partition tile, computes
        psum[p=(b*K+k), l] = sum_d Wsel[d, p] * embT[d, l]
    with Wsel[d, b*K+k] = fc_W[k, d] (host-built constant), then the
    PSUM->SBUF copy fuses  relu(psum + e[p, c])  via ScalarE activation
    bias / VectorE tensor_scalar, and DMA writes a [B, C, K, Lc] output.

v3: all-bf16 datapath (fp32 PSUM accumulation), e applied as per-partition
bias on the copy (so the main matmuls depend only on wsel -- tiny prologue),
paired 2-bank PSUM tiles (one copy per 1024 cols, amortizing the per-op
overhead), DMA ring separation (loads on sync HWDGE, stores alternating
gpsimd SWDGE / scalar HWDGE).  Host upcasts the bf16 output to fp32.
"""

import numpy as np

DEFAULT_CFG = dict(B=64, C=23, L=26000, D=64, E=512, K=3, NCORES=8)

_CACHE = {}


def _derived(cfg):
    B, C, L, D, E, K, NCORES = (cfg[k] for k in ("B", "C", "L", "D", "E", "K", "NCORES"))
    d = dict(cfg)
    d["LC"] = L // NCORES
    d["BC"] = B * C
    d["EP"] = min(128, E)              # contract chunk for eos matmul
    assert E % d["EP"] == 0
    d["NQ"] = E // d["EP"]
    d["ROWS"] = K * B                  # output partition rows (b*K + k)
    # partition tiles over ROWS: cut at b boundaries so each tile's DMA rows
    # merge into contiguous [K*LC] runs per b
    tiles = []
    bmax = 128 // K                    # b's per tile
    b0 = 0
    while b0 < B:
        nb = min(bmax, B - b0)
        tiles.append((b0 * K, nb * K, b0, nb))
        b0 += nb
    d["PTILES"] = tiles                # (p_off, p_n, b0, nb)
    # free-dim chunks: pairs of 512-col matmuls share a 2-bank PSUM tile
    fc = min(1024, d["LC"])
    d["NF"] = [fc] * (d["LC"] // fc) + ([d["LC"] % fc] if d["LC"] % fc else [])
    return d


def _build_nc(cfg=None):
    import concourse.bass as bass  # noqa: F401
    import concourse.mybir as mybir
    import concourse.tile as tile
    from concourse import bacc

    g = _derived(cfg or DEFAULT_CFG)
    B, C, D, K = g["B"], g["C"], g["D"], g["K"]
    LC, BC, EP, NQ, ROWS = g["LC"], g["BC"], g["EP"], g["NQ"], g["ROWS"]
    FCH = min(512, BC)

    f32 = mybir.dt.float32
    bf16 = mybir.dt.bfloat16
    add_op = mybir.AluOpType.add
    max_op = mybir.AluOpType.max

    nc = bacc.Bacc(None)

    embT = nc.declare_dram_parameter("embT", [D, C * LC], bf16, isOutput=False)
    eosE = nc.declare_dram_parameter("eosE", [EP, NQ * BC], bf16, isOutput=False)
    eosW = nc.declare_dram_parameter("eosW", [EP, NQ * D], bf16, isOutput=False)
    WeT = nc.declare_dram_parameter("WeT", [D, K], bf16, isOutput=False)
    eos_b = nc.declare_dram_parameter("eos_b", [D, 1], f32, isOutput=False)
    fc_b = nc.declare_dram_parameter("fc_b", [K, 1], f32, isOutput=False)
    wsel = nc.declare_dram_parameter("wsel", [D, C * ROWS], bf16, isOutput=False)
    out = nc.declare_dram_parameter("out", [B, C, K, LC], bf16, isOutput=True)

    with tile.TileContext(nc) as tc:
        with (
            tc.tile_pool(name="consts", bufs=1) as consts,
            tc.tile_pool(name="setup_sb", bufs=1) as setup_sb,
            tc.tile_pool(name="dscr", bufs=1, space="DRAM") as dscr,
            tc.tile_pool(name="setup_ps", bufs=1, space="PSUM") as setup_ps,
            tc.tile_pool(name="emb", bufs=4) as emb_pool,
            tc.tile_pool(name="osb", bufs=6) as osb_pool,
            tc.tile_pool(name="ops", bufs=3, space="PSUM") as ops_pool,
        ):
            # ---- constants / setup -------------------------------------
            # wsel first (scalar HWDGE): it alone gates the main matmuls
            se = consts.tile([D, C * ROWS], bf16)
            nc.scalar.dma_start(se[:, :], wsel[:, :])

            eosE_sb = setup_sb.tile([EP, NQ * BC], bf16)
            for q in range(NQ):
                nc.gpsimd.dma_start(
                    eosE_sb[:, q * BC:(q + 1) * BC],
                    eosE[:, q * BC:(q + 1) * BC])
            eosW_sb = setup_sb.tile([EP, NQ * D], bf16)
            nc.gpsimd.dma_start(eosW_sb[:, :], eosW[:, :])
            WeT_sb = setup_sb.tile([D, K], bf16)
            nc.sync.dma_start(WeT_sb[:, :], WeT[:, :])
            eosb_sb = setup_sb.tile([D, 1], f32)
            nc.sync.dma_start(eosb_sb[:, :], eos_b[:, :])
            fcb_sb = setup_sb.tile([K, 1], f32)
            nc.sync.dma_start(fcb_sb[:, :], fc_b[:, :])

            # eosT[d, bc] = sum_E eos_W[d, E] * eos_emb[bc, E]  (+ eos_b)
            eosT_sb = setup_sb.tile([D, BC], bf16)
            bc_chunks = [(i, min(FCH, BC - i)) for i in range(0, BC, FCH)]
            for bc0, nbc in bc_chunks:
                eosT_ps = setup_ps.tile([D, nbc], f32, tag="eos_ps")
                for q in range(NQ):
                    nc.tensor.matmul(
                        eosT_ps[:, :],
                        lhsT=eosW_sb[:, q * D:(q + 1) * D],
                        rhs=eosE_sb[:, q * BC + bc0: q * BC + bc0 + nbc],
                        start=(q == 0),
                        stop=(q == NQ - 1),
                    )
                nc.scalar.add(eosT_sb[:, bc0:bc0 + nbc], eosT_ps[:, :], eosb_sb[:, 0:1])

            # e_fold[k, (b,c)] = sum_d We[d,k] * eosT[d, (b,c)]  + fc_b[k]
            e_sb = setup_sb.tile([K, BC], f32)
            for bc0, nbc in bc_chunks:
                e_ps = setup_ps.tile([K, nbc], f32, tag="eos_ps")
                nc.tensor.matmul(
                    e_ps[:, :],
                    lhsT=WeT_sb[:, :],
                    rhs=eosT_sb[:, bc0:bc0 + nbc],
                    start=True,
                    stop=True,
                )
                nc.scalar.add(e_sb[:, bc0:bc0 + nbc], e_ps[:, :], fcb_sb[:, 0:1])
            # scatter e_fold[k, (b,c)] -> eCol[(b*K+k), c] via a DRAM
            # round-trip (the DRAM tile gives arbitrary re-indexing; the
            # tile pool tracks the W->R dependency)
            eDram = dscr.tile([ROWS, C], f32)        # [(b*K+k), c] layout
            nc.sync.dma_start(
                eDram[:, :].rearrange("(b k) c -> k b c", b=B, k=K),
                e_sb[:, :].rearrange("k (b c) -> k b c", b=B, c=C),
            )
            eCols = []
            for (p_off, p_n, b0, nb) in g["PTILES"]:
                eC = consts.tile([p_n, C], f32, tag=f"eCol{p_off}")
                nc.sync.dma_start(eC[:, :], eDram[p_off:p_off + p_n, :])
                eCols.append(eC)

            # ---- main loop ---------------------------------------------
            # DMA ring assignment: embT loads ride the sync HWDGE ring so
            # they are never queued behind output stores; stores alternate
            # gpsimd (SWDGE) and scalar (HWDGE) rings.
            out_bkl = out.rearrange("b c k l -> c b (k l)")
            store_engines = [nc.gpsimd, nc.scalar]
            n_dma = 0
            ncopy = 0
            for c in range(C):
                et = emb_pool.tile([D, LC], bf16, tag="embT")
                nc.sync.dma_start(et[:, :], embT[:, c * LC:(c + 1) * LC])
                for ti, (p_off, p_n, b0, nb) in enumerate(g["PTILES"]):
                    so = osb_pool.tile([p_n, LC], bf16, tag="out_sb")
                    bias = eCols[ti][:, c:c + 1]
                    f0 = 0
                    for nf in g["NF"]:
                        po = ops_pool.tile([p_n, 1024], f32, tag="out_ps")
                        for h0 in range(0, nf, 512):
                            hn = min(512, nf - h0)
                            nc.tensor.matmul(
                                po[:, h0:h0 + hn],
                                lhsT=se[:, c * ROWS + p_off: c * ROWS + p_off + p_n],
                                rhs=et[:, f0 + h0:f0 + h0 + hn],
                                start=True,
                                stop=True,
                            )
                        # fused  relu(psum + e)  on the PSUM->SBUF copy;
                        # alternate engines
                        if ncopy % 2 == 0:
                            nc.scalar.activation(
                                so[:, f0:f0 + nf], po[:, 0:nf],
                                mybir.ActivationFunctionType.Relu,
                                bias=bias,
                            )
                        else:
                            nc.vector.tensor_scalar(
                                so[:, f0:f0 + nf], po[:, 0:nf],
                                bias, 0.0, add_op, max_op,
                            )
                        ncopy += 1
                        f0 += nf
                    store_engines[n_dma % len(store_engines)].dma_start(
                        out_bkl[c, b0:b0 + nb, :], so[:, :]
                    )
                    n_dma += 1
    nc.finalize()
    return nc


def _host_prep(eos_emb, bin_ids, emb_table, eos_W, eos_b, fc_W, fc_b, cfg=None):
    """Build the per-core input maps."""
    import ml_dtypes

    bf16 = ml_dtypes.bfloat16
    g = _derived(cfg or DEFAULT_CFG)
    B, C, L, D, E, K = g["B"], g["C"], g["L"], g["D"], g["E"], g["K"]
    NCORES, LC, BC, EP, NQ, ROWS = (
        g["NCORES"], g["LC"], g["BC"], g["EP"], g["NQ"], g["ROWS"])

    eos_emb = np.ascontiguousarray(eos_emb, dtype=np.float32)
    emb_table = np.ascontiguousarray(emb_table, dtype=np.float32)
    bin_ids = np.asarray(bin_ids)

    # gather (identity when bin_ids == arange, which is the spec'd fill)
    V = C * L
    flat_ids = bin_ids.reshape(-1)
    if flat_ids.shape[0] == V and emb_table.shape[0] == V and \
            flat_ids[0] == 0 and flat_ids[-1] == V - 1 and \
            np.array_equal(flat_ids, np.arange(V, dtype=flat_ids.dtype)):
        bin_emb = emb_table.reshape(C, L, D)
    else:
        bin_emb = emb_table[bin_ids.reshape(C, L)]

    eosE = np.ascontiguousarray(
        eos_emb.reshape(BC, E).T.reshape(NQ, EP, BC).transpose(1, 0, 2).reshape(EP, NQ * BC)
    ).astype(bf16)
    eosW = np.ascontiguousarray(
        np.asarray(eos_W, np.float32).T.reshape(NQ, EP, D).transpose(1, 0, 2).reshape(EP, NQ * D)
    ).astype(bf16)
    fc_W = np.asarray(fc_W, np.float32)
    WeT = np.ascontiguousarray(fc_W[:, D:].T).astype(bf16)   # [D, K]
    eos_b_in = np.asarray(eos_b, np.float32).reshape(D, 1)
    fc_b_in = np.asarray(fc_b, np.float32).reshape(K, 1)
    # wsel[d, c*ROWS + b*K + k] = fc_W[k, d]
    wsel1 = np.tile(fc_W[:, :D], (B, 1)).T               # [D, B*K] (b-major)
    wsel = np.ascontiguousarray(np.tile(wsel1, (1, C))).astype(bf16)

    shared = dict(eosE=eosE, eosW=eosW, WeT=WeT, eos_b=eos_b_in, fc_b=fc_b_in, wsel=wsel)

    in_maps = []
    for i in range(NCORES):
        sl = bin_emb[:, i * LC:(i + 1) * LC, :]          # [C, Lc, D]
        embT_i = np.ascontiguousarray(
            sl.transpose(2, 0, 1).reshape(D, C * LC)).astype(bf16)
        in_maps.append({"embT": embT_i, **shared})
    return in_maps


def _assemble(results, cfg=None):
    g = _derived(cfg or DEFAULT_CFG)
    B, C, L, K, NCORES, LC = g["B"], g["C"], g["L"], g["K"], g["NCORES"], g["LC"]
    out = np.empty((B, C, L, K), np.float32)
    for i in range(NCORES):
        r = results[i]["out"]                            # [B, C, K, Lc] bf16
        out[:, :, i * LC:(i + 1) * LC, :] = r.transpose(0, 1, 3, 2).astype(np.float32)
    return out


def kernel(eos_emb, bin_ids, emb_table, eos_W, eos_b, fc_W, fc_b):
    from concourse.bass_utils import run_bass_kernel_spmd

    if "nc" not in _CACHE:
        _CACHE["nc"] = _build_nc()
    nc = _CACHE["nc"]
    in_maps = _host_prep(eos_emb, bin_ids, emb_table, eos_W, eos_b, fc_W, fc_b)
    res = run_bass_kernel_spmd(nc, in_maps, core_ids=list(range(DEFAULT_CFG["NCORES"])))
    return _assemble(res.results)


# revision 9
# speedup vs baseline: 2.2600x; 1.2906x over previous
"""Trainium2 Bass kernel for nn_Decoder_46660524704357.

Reference computation (shapes hardcoded in DEFAULT_CFG):
    B, C, L, D, E, K = 64, 23, 26000, 64, 512, 3
    eos  = eos_emb @ eos_W.T + eos_b          # [B,C,D]
    bin_emb = emb_table[bin_ids]              # [C,L,D]
    a = bin_emb @ Wb.T                        # [C,L,K]   Wb = fc_W[:, :D]
    e = eos @ We.T + fc_b                     # [B,C,K]   We = fc_W[:, D:]
    out = relu(a[None,:,:,:] + e[:,:,None,:]) # [B,C,L,K]

Sharding: split L across the 8 cores (Lc = 3250 each).  Each core:
  - computes e[B,C,K] on-device via two small matmul chains,
  - for each chromosome c and partition tile, computes
        psum[p=(b*K+k), l] = sum_d Wsel[d, p] * embT[d, l]
    with Wsel[d, b*K+k] = fc_W[k, d] (host-built constant), then the
    PSUM->SBUF copy fuses  relu(psum + e[p, c])  via ScalarE activation
    bias / VectorE tensor_scalar, and DMA writes a [B, C, K, Lc] output.

v3: all-bf16 datapath (fp32 PSUM accumulation), e applied as per-partition
bias on the copy (so the main matmuls depend only on wsel -- tiny prologue),
paired 2-bank PSUM tiles (one copy per 1024 cols, amortizing the per-op
overhead), DMA ring separation (loads on sync HWDGE, stores alternating
gpsimd SWDGE / scalar HWDGE).  Host upcasts the bf16 output to fp32.
"""

import numpy as np

DEFAULT_CFG = dict(B=64, C=23, L=26000, D=64, E=512, K=3, NCORES=8)

_CACHE = {}


def _derived(cfg):
    B, C, L, D, E, K, NCORES = (cfg[k] for k in ("B", "C", "L", "D", "E", "K", "NCORES"))
    d = dict(cfg)
    d["LC"] = L // NCORES
    d["BC"] = B * C
    d["EP"] = min(128, E)              # contract chunk for eos matmul
    assert E % d["EP"] == 0
    d["NQ"] = E // d["EP"]
    d["ROWS"] = K * B                  # output partition rows (b*K + k)
    # partition tiles over ROWS: cut at b boundaries so each tile's DMA rows
    # merge into contiguous [K*LC] runs per b
    tiles = []
    bmax = 128 // K                    # b's per tile
    b0 = 0
    while b0 < B:
        nb = min(bmax, B - b0)
        tiles.append((b0 * K, nb * K, b0, nb))
        b0 += nb
    d["PTILES"] = tiles                # (p_off, p_n, b0, nb)
    # free-dim chunks: pairs of 512-col matmuls share a 2-bank PSUM tile
    fc = min(1024, d["LC"])
    d["NF"] = [fc] * (d["LC"] // fc) + ([d["LC"] % fc] if d["LC"] % fc else [])
    return d


def _build_nc(cfg=None):
    import concourse.bass as bass  # noqa: F401
    import concourse.mybir as mybir
    import concourse.tile as tile
    from concourse import bacc

    g = _derived(cfg or DEFAULT_CFG)
    B, C, D, K = g["B"], g["C"], g["D"], g["K"]
    LC, BC, EP, NQ, ROWS = g["LC"], g["BC"], g["EP"], g["NQ"], g["ROWS"]
    FCH = min(512, BC)

    f32 = mybir.dt.float32
    bf16 = mybir.dt.bfloat16
    fp8 = mybir.dt.float8e4
    add_op = mybir.AluOpType.add
    max_op = mybir.AluOpType.max

    nc = bacc.Bacc(None)

    # embT is fp8 scaled x32 on the host; wsel is bf16 scaled /32, so the
    # mixed-dtype matmul psum comes out unscaled.
    embT = nc.declare_dram_parameter("embT", [D, C * LC], fp8, isOutput=False)
    eosE = nc.declare_dram_parameter("eosE", [EP, NQ * BC], bf16, isOutput=False)
    eosW = nc.declare_dram_parameter("eosW", [EP, NQ * D], bf16, isOutput=False)
    WeT = nc.declare_dram_parameter("WeT", [D, K], bf16, isOutput=False)
    eos_b = nc.declare_dram_parameter("eos_b", [D, 1], f32, isOutput=False)
    fc_b = nc.declare_dram_parameter("fc_b", [K, 1], f32, isOutput=False)
    wsel = nc.declare_dram_parameter("wsel", [D, C * ROWS], bf16, isOutput=False)
    out = nc.declare_dram_parameter("out", [B, C, K, LC], bf16, isOutput=True)

    with tile.TileContext(nc) as tc:
        with (
            tc.tile_pool(name="consts", bufs=1) as consts,
            tc.tile_pool(name="setup_sb", bufs=1) as setup_sb,
            tc.tile_pool(name="dscr", bufs=1, space="DRAM") as dscr,
            tc.tile_pool(name="setup_ps", bufs=1, space="PSUM") as setup_ps,
            tc.tile_pool(name="emb", bufs=4) as emb_pool,
            tc.tile_pool(name="osb", bufs=6) as osb_pool,
            tc.tile_pool(name="ops", bufs=3, space="PSUM") as ops_pool,
        ):
            # ---- constants / setup -------------------------------------
            # wsel first (scalar HWDGE): it alone gates the main matmuls
            se = consts.tile([D, C * ROWS], bf16)
            nc.scalar.dma_start(se[:, :], wsel[:, :])

            eosE_sb = setup_sb.tile([EP, NQ * BC], bf16)
            for q in range(NQ):
                nc.gpsimd.dma_start(
                    eosE_sb[:, q * BC:(q + 1) * BC],
                    eosE[:, q * BC:(q + 1) * BC])
            eosW_sb = setup_sb.tile([EP, NQ * D], bf16)
            nc.gpsimd.dma_start(eosW_sb[:, :], eosW[:, :])
            WeT_sb = setup_sb.tile([D, K], bf16)
            nc.sync.dma_start(WeT_sb[:, :], WeT[:, :])
            eosb_sb = setup_sb.tile([D, 1], f32)
            nc.sync.dma_start(eosb_sb[:, :], eos_b[:, :])
            fcb_sb = setup_sb.tile([K, 1], f32)
            nc.sync.dma_start(fcb_sb[:, :], fc_b[:, :])

            # eosT[d, bc] = sum_E eos_W[d, E] * eos_emb[bc, E]  (+ eos_b)
            eosT_sb = setup_sb.tile([D, BC], bf16)
            bc_chunks = [(i, min(FCH, BC - i)) for i in range(0, BC, FCH)]
            for bc0, nbc in bc_chunks:
                eosT_ps = setup_ps.tile([D, nbc], f32, tag="eos_ps")
                for q in range(NQ):
                    nc.tensor.matmul(
                        eosT_ps[:, :],
                        lhsT=eosW_sb[:, q * D:(q + 1) * D],
                        rhs=eosE_sb[:, q * BC + bc0: q * BC + bc0 + nbc],
                        start=(q == 0),
                        stop=(q == NQ - 1),
                    )
                nc.scalar.add(eosT_sb[:, bc0:bc0 + nbc], eosT_ps[:, :], eosb_sb[:, 0:1])

            # e_fold[k, (b,c)] = sum_d We[d,k] * eosT[d, (b,c)]  + fc_b[k]
            e_sb = setup_sb.tile([K, BC], f32)
            for bc0, nbc in bc_chunks:
                e_ps = setup_ps.tile([K, nbc], f32, tag="eos_ps")
                nc.tensor.matmul(
                    e_ps[:, :],
                    lhsT=WeT_sb[:, :],
                    rhs=eosT_sb[:, bc0:bc0 + nbc],
                    start=True,
                    stop=True,
                )
                nc.scalar.add(e_sb[:, bc0:bc0 + nbc], e_ps[:, :], fcb_sb[:, 0:1])
            # scatter e_fold[k, (b,c)] -> eCol[(b*K+k), c] via a DRAM
            # round-trip (the DRAM tile gives arbitrary re-indexing; the
            # tile pool tracks the W->R dependency)
            eDram = dscr.tile([ROWS, C], f32)        # [(b*K+k), c] layout
            nc.sync.dma_start(
                eDram[:, :].rearrange("(b k) c -> k b c", b=B, k=K),
                e_sb[:, :].rearrange("k (b c) -> k b c", b=B, c=C),
            )
            eCols = []
            for (p_off, p_n, b0, nb) in g["PTILES"]:
                eC = consts.tile([p_n, C], f32, tag=f"eCol{p_off}")
                nc.sync.dma_start(eC[:, :], eDram[p_off:p_off + p_n, :])
                eCols.append(eC)

            # ---- main loop ---------------------------------------------
            # DMA ring assignment: embT loads ride the sync HWDGE ring so
            # they are never queued behind output stores; stores alternate
            # gpsimd (SWDGE) and scalar (HWDGE) rings.
            #
            # The matmul result a[k,l] (at partition p it is a[p%3, l]) is
            # independent of b, so ONE 126-row matmul serves BOTH partition
            # tiles: ptile1's rows (126..191) read the same PSUM rows 0..65
            # (row alignment holds because 126 % 3 == 0) with its own e
            # bias column.  This halves PE streaming work.
            out_bkl = out.rearrange("b c k l -> c b (k l)")
            store_engines = [nc.gpsimd, nc.scalar]
            (p_off0, p_n0, b00, nb0), (p_off1, p_n1, b01, nb1) = g["PTILES"]
            n_dma = 0
            for c in range(C):
                if c % 2 == 0:
                    ncpair = min(2, C - c)
                    et2 = emb_pool.tile([D, ncpair * LC], fp8, tag="embT")
                    nc.sync.dma_start(
                        et2[:, :], embT[:, c * LC:(c + ncpair) * LC])
                et = et2[:, (c % 2) * LC:(c % 2 + 1) * LC]
                so0 = osb_pool.tile([p_n0, LC], bf16, tag="out_sb0")
                so1 = osb_pool.tile([p_n1, LC], bf16, tag="out_sb1")
                bias0 = eCols[0][:, c:c + 1]
                bias1 = eCols[1][:, c:c + 1]
                f0 = 0
                for fi, nf in enumerate(g["NF"]):
                    po = ops_pool.tile([p_n0, 1024], f32, tag="out_ps")
                    for h0 in range(0, nf, 512):
                        hn = min(512, nf - h0)
                        nc.tensor.matmul(
                            po[:, h0:h0 + hn],
                            lhsT=se[:, c * ROWS: c * ROWS + p_n0],
                            rhs=et[:, f0 + h0:f0 + h0 + hn],
                            start=True,
                            stop=True,
                        )
                    # two fused relu(psum + e) copies off the same PSUM
                    # tile, one per partition tile, on different engines
                    if fi % 2 == 0:
                        nc.scalar.activation(
                            so0[:, f0:f0 + nf], po[:, 0:nf],
                            mybir.ActivationFunctionType.Relu,
                            bias=bias0,
                        )
                        nc.vector.tensor_scalar(
                            so1[:, f0:f0 + nf], po[0:p_n1, 0:nf],
                            bias1, 0.0, add_op, max_op,
                        )
                    else:
                        nc.vector.tensor_scalar(
                            so0[:, f0:f0 + nf], po[:, 0:nf],
                            bias0, 0.0, add_op, max_op,
                        )
                        nc.scalar.activation(
                            so1[:, f0:f0 + nf], po[0:p_n1, 0:nf],
                            mybir.ActivationFunctionType.Relu,
                            bias=bias1,
                        )
                    f0 += nf
                store_engines[n_dma % 2].dma_start(
                    out_bkl[c, b00:b00 + nb0, :], so0[:, :])
                n_dma += 1
                store_engines[n_dma % 2].dma_start(
                    out_bkl[c, b01:b01 + nb1, :], so1[:, :])
                n_dma += 1
    nc.finalize()
    return nc


def _host_prep(eos_emb, bin_ids, emb_table, eos_W, eos_b, fc_W, fc_b, cfg=None):
    """Build the per-core input maps."""
    import ml_dtypes

    bf16 = ml_dtypes.bfloat16
    g = _derived(cfg or DEFAULT_CFG)
    B, C, L, D, E, K = g["B"], g["C"], g["L"], g["D"], g["E"], g["K"]
    NCORES, LC, BC, EP, NQ, ROWS = (
        g["NCORES"], g["LC"], g["BC"], g["EP"], g["NQ"], g["ROWS"])

    eos_emb = np.ascontiguousarray(eos_emb, dtype=np.float32)
    emb_table = np.ascontiguousarray(emb_table, dtype=np.float32)
    bin_ids = np.asarray(bin_ids)

    # gather (identity when bin_ids == arange, which is the spec'd fill)
    V = C * L
    flat_ids = bin_ids.reshape(-1)
    if flat_ids.shape[0] == V and emb_table.shape[0] == V and \
            flat_ids[0] == 0 and flat_ids[-1] == V - 1 and \
            np.array_equal(flat_ids, np.arange(V, dtype=flat_ids.dtype)):
        bin_emb = emb_table.reshape(C, L, D)
    else:
        bin_emb = emb_table[bin_ids.reshape(C, L)]

    eosE = np.ascontiguousarray(
        eos_emb.reshape(BC, E).T.reshape(NQ, EP, BC).transpose(1, 0, 2).reshape(EP, NQ * BC)
    ).astype(bf16)
    eosW = np.ascontiguousarray(
        np.asarray(eos_W, np.float32).T.reshape(NQ, EP, D).transpose(1, 0, 2).reshape(EP, NQ * D)
    ).astype(bf16)
    fc_W = np.asarray(fc_W, np.float32)
    WeT = np.ascontiguousarray(fc_W[:, D:].T).astype(bf16)   # [D, K]
    eos_b_in = np.asarray(eos_b, np.float32).reshape(D, 1)
    fc_b_in = np.asarray(fc_b, np.float32).reshape(K, 1)
    # wsel[d, c*ROWS + b*K + k] = fc_W[k, d] / 32  (embT carries the x32)
    wsel1 = np.tile(fc_W[:, :D] / 32.0, (B, 1)).T        # [D, B*K] (b-major)
    wsel = np.ascontiguousarray(np.tile(wsel1, (1, C))).astype(bf16)

    shared = dict(eosE=eosE, eosW=eosW, WeT=WeT, eos_b=eos_b_in, fc_b=fc_b_in, wsel=wsel)

    import concourse.mybir as mybir

    fp8 = mybir.dt.np(mybir.dt.float8e4)
    in_maps = []
    for i in range(NCORES):
        sl = bin_emb[:, i * LC:(i + 1) * LC, :]          # [C, Lc, D]
        embT_i = np.ascontiguousarray(
            sl.transpose(2, 0, 1).reshape(D, C * LC) * np.float32(32.0)
        ).astype(fp8)
        in_maps.append({"embT": embT_i, **shared})
    return in_maps


def _assemble(results, cfg=None):
    g = _derived(cfg or DEFAULT_CFG)
    B, C, L, K, NCORES, LC = g["B"], g["C"], g["L"], g["K"], g["NCORES"], g["LC"]
    out = np.empty((B, C, L, K), np.float32)
    for i in range(NCORES):
        r = results[i]["out"]                            # [B, C, K, Lc] bf16
        out[:, :, i * LC:(i + 1) * LC, :] = r.transpose(0, 1, 3, 2).astype(np.float32)
    return out


def kernel(eos_emb, bin_ids, emb_table, eos_W, eos_b, fc_W, fc_b):
    from concourse.bass_utils import run_bass_kernel_spmd

    if "nc" not in _CACHE:
        _CACHE["nc"] = _build_nc()
    nc = _CACHE["nc"]
    in_maps = _host_prep(eos_emb, bin_ids, emb_table, eos_W, eos_b, fc_W, fc_b)
    res = run_bass_kernel_spmd(nc, in_maps, core_ids=list(range(DEFAULT_CFG["NCORES"])))
    return _assemble(res.results)


# revision 13
# speedup vs baseline: 2.2731x; 1.0058x over previous
"""Trainium2 Bass kernel for nn_Decoder_46660524704357.

Reference computation (shapes hardcoded in DEFAULT_CFG):
    B, C, L, D, E, K = 64, 23, 26000, 64, 512, 3
    eos  = eos_emb @ eos_W.T + eos_b          # [B,C,D]
    bin_emb = emb_table[bin_ids]              # [C,L,D]
    a = bin_emb @ Wb.T                        # [C,L,K]   Wb = fc_W[:, :D]
    e = eos @ We.T + fc_b                     # [B,C,K]   We = fc_W[:, D:]
    out = relu(a[None,:,:,:] + e[:,:,None,:]) # [B,C,L,K]

Sharding: split L across the 8 cores (Lc = 3250 each).  Each core:
  - computes e[B,C,K] on-device via two small matmul chains,
  - for each chromosome c and partition tile, computes
        psum[p=(b*K+k), l] = sum_d Wsel[d, p] * embT[d, l]
    with Wsel[d, b*K+k] = fc_W[k, d] (host-built constant), then the
    PSUM->SBUF copy fuses  relu(psum + e[p, c])  via ScalarE activation
    bias / VectorE tensor_scalar, and DMA writes a [B, C, K, Lc] output.

v3: all-bf16 datapath (fp32 PSUM accumulation), e applied as per-partition
bias on the copy (so the main matmuls depend only on wsel -- tiny prologue),
paired 2-bank PSUM tiles (one copy per 1024 cols, amortizing the per-op
overhead), DMA ring separation (loads on sync HWDGE, stores alternating
gpsimd SWDGE / scalar HWDGE).  Host upcasts the bf16 output to fp32.
"""

import numpy as np

DEFAULT_CFG = dict(B=64, C=23, L=26000, D=64, E=512, K=3, NCORES=8)

_CACHE = {}


def _derived(cfg):
    B, C, L, D, E, K, NCORES = (cfg[k] for k in ("B", "C", "L", "D", "E", "K", "NCORES"))
    d = dict(cfg)
    d["LC"] = L // NCORES
    d["BC"] = B * C
    d["EP"] = min(128, E)              # contract chunk for eos matmul
    assert E % d["EP"] == 0
    d["NQ"] = E // d["EP"]
    d["ROWS"] = K * B                  # output partition rows (b*K + k)
    # partition tiles over ROWS: cut at b boundaries so each tile's DMA rows
    # merge into contiguous [K*LC] runs per b
    tiles = []
    bmax = 128 // K                    # b's per tile
    b0 = 0
    while b0 < B:
        nb = min(bmax, B - b0)
        tiles.append((b0 * K, nb * K, b0, nb))
        b0 += nb
    d["PTILES"] = tiles                # (p_off, p_n, b0, nb)
    # free-dim chunks: pairs of 512-col matmuls share a 2-bank PSUM tile
    fc = min(1024, d["LC"])
    d["NF"] = [fc] * (d["LC"] // fc) + ([d["LC"] % fc] if d["LC"] % fc else [])
    return d


def _build_nc(cfg=None):
    import concourse.bass as bass  # noqa: F401
    import concourse.mybir as mybir
    import concourse.tile as tile
    from concourse import bacc

    g = _derived(cfg or DEFAULT_CFG)
    B, C, D, K = g["B"], g["C"], g["D"], g["K"]
    LC, BC, EP, NQ, ROWS = g["LC"], g["BC"], g["EP"], g["NQ"], g["ROWS"]
    FCH = min(512, BC)

    f32 = mybir.dt.float32
    bf16 = mybir.dt.bfloat16
    fp8 = mybir.dt.float8e4
    add_op = mybir.AluOpType.add
    max_op = mybir.AluOpType.max

    nc = bacc.Bacc(None)

    # embT is fp8 scaled x32 on the host; wsel is bf16 scaled /32, so the
    # mixed-dtype matmul psum comes out unscaled.
    embT = nc.declare_dram_parameter("embT", [D, C * LC], fp8, isOutput=False)
    eosE = nc.declare_dram_parameter("eosE", [EP, NQ * BC], bf16, isOutput=False)
    # W2[k,E] = (We @ eos_W)[k,E] and bias2 = We@eos_b + fc_b are host-folded
    # (weights-only preprocessing), so e = W2 @ eos_emb^T + bias2 is a single
    # on-device matmul stage.
    W2T = nc.declare_dram_parameter("W2T", [EP, NQ * K], bf16, isOutput=False)
    bias2 = nc.declare_dram_parameter("bias2", [K, 1], f32, isOutput=False)
    wsel = nc.declare_dram_parameter("wsel", [D, C * ROWS], bf16, isOutput=False)
    out = nc.declare_dram_parameter("out", [B, C, K, LC], bf16, isOutput=True)

    with tile.TileContext(nc) as tc:
        with (
            tc.tile_pool(name="consts", bufs=1) as consts,
            tc.tile_pool(name="setup_sb", bufs=1) as setup_sb,
            tc.tile_pool(name="dscr", bufs=1, space="DRAM") as dscr,
            tc.tile_pool(name="setup_ps", bufs=1, space="PSUM") as setup_ps,
            tc.tile_pool(name="emb", bufs=3) as emb_pool,
            tc.tile_pool(name="osb", bufs=6) as osb_pool,
            tc.tile_pool(name="ops", bufs=3, space="PSUM") as ops_pool,
            tc.tile_pool(name="warm", bufs=1, space="PSUM") as warm_pool,
        ):
            # ---- PE warm-up --------------------------------------------
            # A dense accumulation chain (no inter-MM hazards) to trip the
            # HAM clock gate (cold 1.2 GHz -> warm 2.4 GHz takes ~3.4us of
            # sustained PE activity) while the setup DMAs are in flight.
            wu = setup_sb.tile([D, 512], bf16)
            nc.vector.memset(wu[:, :], 0.001)
            wu_ps = warm_pool.tile([126, 512], f32)
            NWARM = 12
            for i in range(NWARM):
                nc.tensor.matmul(
                    wu_ps[:, :], lhsT=wu[:, 0:126], rhs=wu[:, :],
                    start=(i == 0), stop=(i == NWARM - 1),
                )
            wu_junk = setup_sb.tile([1, 1], f32)
            nc.vector.tensor_scalar_max(wu_junk[0:1, 0:1], wu_ps[0:1, 0:1], 0.0)

            # ---- constants / setup -------------------------------------
            # wsel first (scalar HWDGE): it alone gates the main matmuls
            se = consts.tile([D, C * ROWS], bf16)
            nc.scalar.dma_start(se[:, :], wsel[:, :])

            eosE_sb = setup_sb.tile([EP, NQ * BC], bf16)
            for q in range(NQ):
                nc.gpsimd.dma_start(
                    eosE_sb[:, q * BC:(q + 1) * BC],
                    eosE[:, q * BC:(q + 1) * BC])
            W2T_sb = setup_sb.tile([EP, NQ * K], bf16)
            nc.sync.dma_start(W2T_sb[:, :], W2T[:, :])
            b2_sb = setup_sb.tile([K, 1], f32)
            nc.sync.dma_start(b2_sb[:, :], bias2[:, :])

            # e_fold[k, (b,c)] = sum_E W2[k,E] * eos_emb[(b,c),E]  + bias2[k]
            e_sb = setup_sb.tile([K, BC], f32)
            bc_chunks = [(i, min(FCH, BC - i)) for i in range(0, BC, FCH)]
            for bc0, nbc in bc_chunks:
                e_ps = setup_ps.tile([K, nbc], f32, tag="eos_ps")
                for q in range(NQ):
                    nc.tensor.matmul(
                        e_ps[:, :],
                        lhsT=W2T_sb[:, q * K:(q + 1) * K],
                        rhs=eosE_sb[:, q * BC + bc0: q * BC + bc0 + nbc],
                        start=(q == 0),
                        stop=(q == NQ - 1),
                    )
                nc.scalar.add(e_sb[:, bc0:bc0 + nbc], e_ps[:, :], b2_sb[:, 0:1])
            # scatter e_fold[k, (b,c)] -> eCol[(b*K+k), c] via a DRAM
            # round-trip (the DRAM tile gives arbitrary re-indexing; the
            # tile pool tracks the W->R dependency)
            eDram = dscr.tile([ROWS, C], f32)        # [(b*K+k), c] layout
            nc.sync.dma_start(
                eDram[:, :].rearrange("(b k) c -> k b c", b=B, k=K),
                e_sb[:, :].rearrange("k (b c) -> k b c", b=B, c=C),
            )
            eCols = []
            for (p_off, p_n, b0, nb) in g["PTILES"]:
                eC = consts.tile([p_n, C], f32, tag=f"eCol{p_off}")
                nc.sync.dma_start(eC[:, :], eDram[p_off:p_off + p_n, :])
                eCols.append(eC)

            # ---- main loop ---------------------------------------------
            # DMA ring assignment: embT loads ride the sync HWDGE ring so
            # they are never queued behind output stores; stores alternate
            # gpsimd (SWDGE) and scalar (HWDGE) rings.
            #
            # The matmul result a[k,l] (at partition p it is a[p%3, l]) is
            # independent of b, so ONE 126-row matmul serves BOTH partition
            # tiles: ptile1's rows (126..191) read the same PSUM rows 0..65
            # (row alignment holds because 126 % 3 == 0) with its own e
            # bias column.  This halves PE streaming work.
            out_bkl = out.rearrange("b c k l -> c b (k l)")
            store_engines = [nc.gpsimd, nc.scalar]
            (p_off0, p_n0, b00, nb0), (p_off1, p_n1, b01, nb1) = g["PTILES"]
            n_dma = 0
            for c in range(C):
                if c % 2 == 0:
                    ncpair = min(2, C - c)
                    et2 = emb_pool.tile([D, ncpair * LC], fp8, tag="embT")
                    nc.sync.dma_start(
                        et2[:, :], embT[:, c * LC:(c + ncpair) * LC])
                et = et2[:, (c % 2) * LC:(c % 2 + 1) * LC]
                so0 = osb_pool.tile([p_n0, LC], bf16, tag="out_sb0")
                so1 = osb_pool.tile([p_n1, LC], bf16, tag="out_sb1")
                bias0 = eCols[0][:, c:c + 1]
                bias1 = eCols[1][:, c:c + 1]
                f0 = 0
                for fi, nf in enumerate(g["NF"]):
                    po = ops_pool.tile([p_n0, 1024], f32, tag="out_ps")
                    for h0 in range(0, nf, 512):
                        hn = min(512, nf - h0)
                        nc.tensor.matmul(
                            po[:, h0:h0 + hn],
                            lhsT=se[:, c * ROWS: c * ROWS + p_n0],
                            rhs=et[:, f0 + h0:f0 + h0 + hn],
                            start=True,
                            stop=True,
                        )
                    # two fused relu(psum + e) copies off the same PSUM
                    # tile, one per partition tile, on different engines
                    if fi % 2 == 0:
                        nc.scalar.activation(
                            so0[:, f0:f0 + nf], po[:, 0:nf],
                            mybir.ActivationFunctionType.Relu,
                            bias=bias0,
                        )
                        nc.vector.tensor_scalar(
                            so1[:, f0:f0 + nf], po[0:p_n1, 0:nf],
                            bias1, 0.0, add_op, max_op,
                        )
                    else:
                        nc.vector.tensor_scalar(
                            so0[:, f0:f0 + nf], po[:, 0:nf],
                            bias0, 0.0, add_op, max_op,
                        )
                        nc.scalar.activation(
                            so1[:, f0:f0 + nf], po[0:p_n1, 0:nf],
                            mybir.ActivationFunctionType.Relu,
                            bias=bias1,
                        )
                    f0 += nf
                # alternate per c so each ring gets equal bytes (so0 is
                # twice so1's size)
                store_engines[c % 2].dma_start(
                    out_bkl[c, b00:b00 + nb0, :], so0[:, :])
                store_engines[(c + 1) % 2].dma_start(
                    out_bkl[c, b01:b01 + nb1, :], so1[:, :])
    nc.finalize()
    return nc


def _host_prep(eos_emb, bin_ids, emb_table, eos_W, eos_b, fc_W, fc_b, cfg=None):
    """Build the per-core input maps."""
    import ml_dtypes

    bf16 = ml_dtypes.bfloat16
    g = _derived(cfg or DEFAULT_CFG)
    B, C, L, D, E, K = g["B"], g["C"], g["L"], g["D"], g["E"], g["K"]
    NCORES, LC, BC, EP, NQ, ROWS = (
        g["NCORES"], g["LC"], g["BC"], g["EP"], g["NQ"], g["ROWS"])

    eos_emb = np.ascontiguousarray(eos_emb, dtype=np.float32)
    emb_table = np.ascontiguousarray(emb_table, dtype=np.float32)
    bin_ids = np.asarray(bin_ids)

    # gather (identity when bin_ids == arange, which is the spec'd fill)
    V = C * L
    flat_ids = bin_ids.reshape(-1)
    if flat_ids.shape[0] == V and emb_table.shape[0] == V and \
            flat_ids[0] == 0 and flat_ids[-1] == V - 1 and \
            np.array_equal(flat_ids, np.arange(V, dtype=flat_ids.dtype)):
        bin_emb = emb_table.reshape(C, L, D)
    else:
        bin_emb = emb_table[bin_ids.reshape(C, L)]

    eosE = np.ascontiguousarray(
        eos_emb.reshape(BC, E).T.reshape(NQ, EP, BC).transpose(1, 0, 2).reshape(EP, NQ * BC)
    ).astype(bf16)
    fc_W = np.asarray(fc_W, np.float32)
    eos_W = np.asarray(eos_W, np.float32)
    # weights-only folds:  W2 = We @ eos_W  [K, E],  bias2 = We@eos_b + fc_b
    We = fc_W[:, D:]                                     # [K, D]
    W2 = We @ eos_W                                      # [K, E]
    W2T = np.ascontiguousarray(
        W2.T.reshape(NQ, EP, K).transpose(1, 0, 2).reshape(EP, NQ * K)
    ).astype(bf16)
    bias2 = (We @ np.asarray(eos_b, np.float32).reshape(D)
             + np.asarray(fc_b, np.float32)).reshape(K, 1).astype(np.float32)
    # wsel[d, c*ROWS + b*K + k] = fc_W[k, d] / 32  (embT carries the x32)
    wsel1 = np.tile(fc_W[:, :D] / 32.0, (B, 1)).T        # [D, B*K] (b-major)
    wsel = np.ascontiguousarray(np.tile(wsel1, (1, C))).astype(bf16)

    shared = dict(eosE=eosE, W2T=W2T, bias2=bias2, wsel=wsel)

    import concourse.mybir as mybir

    fp8 = mybir.dt.np(mybir.dt.float8e4)
    in_maps = []
    for i in range(NCORES):
        sl = bin_emb[:, i * LC:(i + 1) * LC, :]          # [C, Lc, D]
        embT_i = np.ascontiguousarray(
            sl.transpose(2, 0, 1).reshape(D, C * LC) * np.float32(32.0)
        ).astype(fp8)
        in_maps.append({"embT": embT_i, **shared})
    return in_maps


def _assemble(results, cfg=None):
    g = _derived(cfg or DEFAULT_CFG)
    B, C, L, K, NCORES, LC = g["B"], g["C"], g["L"], g["K"], g["NCORES"], g["LC"]
    out = np.empty((B, C, L, K), np.float32)
    for i in range(NCORES):
        r = results[i]["out"]                            # [B, C, K, Lc] bf16
        out[:, :, i * LC:(i + 1) * LC, :] = r.transpose(0, 1, 3, 2).astype(np.float32)
    return out


def kernel(eos_emb, bin_ids, emb_table, eos_W, eos_b, fc_W, fc_b):
    from concourse.bass_utils import run_bass_kernel_spmd

    if "nc" not in _CACHE:
        _CACHE["nc"] = _build_nc()
    nc = _CACHE["nc"]
    in_maps = _host_prep(eos_emb, bin_ids, emb_table, eos_W, eos_b, fc_W, fc_b)
    res = run_bass_kernel_spmd(nc, in_maps, core_ids=list(range(DEFAULT_CFG["NCORES"])))
    return _assemble(res.results)


# revision 16
# speedup vs baseline: 2.5492x; 1.1214x over previous
"""Trainium2 Bass kernel for nn_Decoder_46660524704357.

Reference computation (shapes hardcoded in DEFAULT_CFG):
    B, C, L, D, E, K = 64, 23, 26000, 64, 512, 3
    eos  = eos_emb @ eos_W.T + eos_b          # [B,C,D]
    bin_emb = emb_table[bin_ids]              # [C,L,D]
    a = bin_emb @ Wb.T                        # [C,L,K]   Wb = fc_W[:, :D]
    e = eos @ We.T + fc_b                     # [B,C,K]   We = fc_W[:, D:]
    out = relu(a[None,:,:,:] + e[:,:,None,:]) # [B,C,L,K]

Sharding: split L across the 8 cores (Lc = 3250 each).  Each core:
  - computes e[B,C,K] on-device via two small matmul chains,
  - for each chromosome c and partition tile, computes
        psum[p=(b*K+k), l] = sum_d Wsel[d, p] * embT[d, l]
    with Wsel[d, b*K+k] = fc_W[k, d] (host-built constant), then the
    PSUM->SBUF copy fuses  relu(psum + e[p, c])  via ScalarE activation
    bias / VectorE tensor_scalar, and DMA writes a [B, C, K, Lc] output.

v3: all-bf16 datapath (fp32 PSUM accumulation), e applied as per-partition
bias on the copy (so the main matmuls depend only on wsel -- tiny prologue),
paired 2-bank PSUM tiles (one copy per 1024 cols, amortizing the per-op
overhead), DMA ring separation (loads on sync HWDGE, stores alternating
gpsimd SWDGE / scalar HWDGE).  Host upcasts the bf16 output to fp32.
"""

import numpy as np

DEFAULT_CFG = dict(B=64, C=23, L=26000, D=64, E=512, K=3, NCORES=8)

_CACHE = {}


def _derived(cfg):
    B, C, L, D, E, K, NCORES = (cfg[k] for k in ("B", "C", "L", "D", "E", "K", "NCORES"))
    d = dict(cfg)
    d["LC"] = L // NCORES
    d["BC"] = B * C
    d["EP"] = min(128, E)              # contract chunk for eos matmul
    assert E % d["EP"] == 0
    d["NQ"] = E // d["EP"]
    d["ROWS"] = K * B                  # output partition rows (b*K + k)
    # partition tiles over ROWS: cut at b boundaries so each tile's DMA rows
    # merge into contiguous [K*LC] runs per b
    tiles = []
    bmax = 128 // K                    # b's per tile
    b0 = 0
    while b0 < B:
        nb = min(bmax, B - b0)
        tiles.append((b0 * K, nb * K, b0, nb))
        b0 += nb
    d["PTILES"] = tiles                # (p_off, p_n, b0, nb)
    # free-dim chunks: pairs of 512-col matmuls share a 2-bank PSUM tile
    fc = min(1024, d["LC"])
    d["NF"] = [fc] * (d["LC"] // fc) + ([d["LC"] % fc] if d["LC"] % fc else [])
    return d


def _build_nc(cfg=None):
    import concourse.bass as bass  # noqa: F401
    import concourse.mybir as mybir
    import concourse.tile as tile
    from concourse import bacc

    g = _derived(cfg or DEFAULT_CFG)
    B, C, D, K = g["B"], g["C"], g["D"], g["K"]
    LC, BC, EP, NQ, ROWS = g["LC"], g["BC"], g["EP"], g["NQ"], g["ROWS"]
    FCH = min(512, BC)

    f32 = mybir.dt.float32
    bf16 = mybir.dt.bfloat16
    fp8 = mybir.dt.float8e4
    add_op = mybir.AluOpType.add
    max_op = mybir.AluOpType.max

    nc = bacc.Bacc(None)

    # embT is fp8 scaled x32 on the host; wsel is bf16 scaled /32, so the
    # mixed-dtype matmul psum comes out unscaled.
    embT = nc.declare_dram_parameter("embT", [D, C * LC], fp8, isOutput=False)
    eosE = nc.declare_dram_parameter("eosE", [EP, NQ * BC], bf16, isOutput=False)
    # W2[k,E] = (We @ eos_W)[k,E] and bias2 = We@eos_b + fc_b are host-folded
    # (weights-only preprocessing), so e = W2 @ eos_emb^T + bias2 is a single
    # on-device matmul stage.
    W2T = nc.declare_dram_parameter("W2T", [EP, NQ * K], bf16, isOutput=False)
    bias2 = nc.declare_dram_parameter("bias2", [K, 1], f32, isOutput=False)
    wsel = nc.declare_dram_parameter("wsel", [D, C * ROWS], bf16, isOutput=False)
    out = nc.declare_dram_parameter("out", [B, C, K, LC], bf16, isOutput=True)

    with tile.TileContext(nc) as tc:
        with (
            tc.tile_pool(name="consts", bufs=1) as consts,
            tc.tile_pool(name="setup_sb", bufs=1) as setup_sb,
            tc.tile_pool(name="dscr", bufs=1, space="DRAM") as dscr,
            tc.tile_pool(name="setup_ps", bufs=1, space="PSUM") as setup_ps,
            tc.tile_pool(name="emb", bufs=3) as emb_pool,
            tc.tile_pool(name="osb", bufs=6) as osb_pool,
            tc.tile_pool(name="ops", bufs=3, space="PSUM") as ops_pool,
        ):
            # ---- constants / setup -------------------------------------
            # wsel first (scalar HWDGE): it alone gates the main matmuls
            se = consts.tile([D, C * ROWS], bf16)
            nc.scalar.dma_start(se[:, :], wsel[:, :])

            eosE_sb = setup_sb.tile([EP, NQ * BC], bf16)
            for q in range(NQ):
                nc.gpsimd.dma_start(
                    eosE_sb[:, q * BC:(q + 1) * BC],
                    eosE[:, q * BC:(q + 1) * BC])
            W2T_sb = setup_sb.tile([EP, NQ * K], bf16)
            nc.scalar.dma_start(W2T_sb[:, :], W2T[:, :])
            b2_sb = setup_sb.tile([K, 1], f32)
            nc.scalar.dma_start(b2_sb[:, :], bias2[:, :])

            # e_fold[k, (b,c)] = sum_E W2[k,E] * eos_emb[(b,c),E]  + bias2[k]
            e_sb = setup_sb.tile([K, BC], f32)
            bc_chunks = [(i, min(FCH, BC - i)) for i in range(0, BC, FCH)]
            for bc0, nbc in bc_chunks:
                e_ps = setup_ps.tile([K, nbc], f32, tag="eos_ps")
                for q in range(NQ):
                    nc.tensor.matmul(
                        e_ps[:, :],
                        lhsT=W2T_sb[:, q * K:(q + 1) * K],
                        rhs=eosE_sb[:, q * BC + bc0: q * BC + bc0 + nbc],
                        start=(q == 0),
                        stop=(q == NQ - 1),
                    )
                nc.scalar.add(e_sb[:, bc0:bc0 + nbc], e_ps[:, :], b2_sb[:, 0:1])
            # scatter e_fold[k, (b,c)] -> eCol[(b*K+k), c] via a DRAM
            # round-trip (the DRAM tile gives arbitrary re-indexing; the
            # tile pool tracks the W->R dependency)
            # scatter on the scalar HWDGE ring: stores haven't started yet
            # (they gate on these), and the sync ring must stay clear
            eDram = dscr.tile([ROWS, C], f32)        # [(b*K+k), c] layout
            nc.scalar.dma_start(
                eDram[:, :].rearrange("(b k) c -> k b c", b=B, k=K),
                e_sb[:, :].rearrange("k (b c) -> k b c", b=B, c=C),
            )
            eCols = []
            for (p_off, p_n, b0, nb) in g["PTILES"]:
                eC = consts.tile([p_n, C], f32, tag=f"eCol{p_off}")
                nc.scalar.dma_start(eC[:, :], eDram[p_off:p_off + p_n, :])
                eCols.append(eC)

            # ---- main loop ---------------------------------------------
            # DMA ring assignment: embT loads ride the sync HWDGE ring so
            # they are never queued behind output stores; stores alternate
            # gpsimd (SWDGE) and scalar (HWDGE) rings.
            #
            # The matmul result a[k,l] (at partition p it is a[p%3, l]) is
            # independent of b, so ONE 126-row matmul serves BOTH partition
            # tiles: ptile1's rows (126..191) read the same PSUM rows 0..65
            # (row alignment holds because 126 % 3 == 0) with its own e
            # bias column.  This halves PE streaming work.
            out_bkl = out.rearrange("b c k l -> c b (k l)")
            (p_off0, p_n0, b00, nb0), (p_off1, p_n1, b01, nb1) = g["PTILES"]
            for c in range(C):
                if c % 2 == 0:
                    ncpair = min(2, C - c)
                    et2 = emb_pool.tile([D, ncpair * LC], fp8, tag="embT")
                    nc.gpsimd.dma_start(
                        et2[:, :], embT[:, c * LC:(c + ncpair) * LC])
                et = et2[:, (c % 2) * LC:(c % 2 + 1) * LC]
                so0 = osb_pool.tile([p_n0, LC], bf16, tag="out_sb0")
                so1 = osb_pool.tile([p_n1, LC], bf16, tag="out_sb1")
                bias0 = eCols[0][:, c:c + 1]
                bias1 = eCols[1][:, c:c + 1]
                f0 = 0
                for fi, nf in enumerate(g["NF"]):
                    po = ops_pool.tile([p_n0, 1024], f32, tag="out_ps")
                    for h0 in range(0, nf, 512):
                        hn = min(512, nf - h0)
                        nc.tensor.matmul(
                            po[:, h0:h0 + hn],
                            lhsT=se[:, c * ROWS: c * ROWS + p_n0],
                            rhs=et[:, f0 + h0:f0 + h0 + hn],
                            start=True,
                            stop=True,
                        )
                    # both fused relu(psum + e) copies of a chunk go to ONE
                    # engine (ScalarE+VectorE can only access PSUM in
                    # parallel on DIFFERENT banks); chunks alternate between
                    # engines, with the pairing flipped every c to balance
                    # columns (chunk sizes are 1024,1024,1024,178)
                    if (fi + c) % 2 == 0:
                        nc.scalar.activation(
                            so0[:, f0:f0 + nf], po[:, 0:nf],
                            mybir.ActivationFunctionType.Relu,
                            bias=bias0,
                        )
                        nc.scalar.activation(
                            so1[:, f0:f0 + nf], po[0:p_n1, 0:nf],
                            mybir.ActivationFunctionType.Relu,
                            bias=bias1,
                        )
                    else:
                        nc.vector.tensor_scalar(
                            so0[:, f0:f0 + nf], po[:, 0:nf],
                            bias0, 0.0, add_op, max_op,
                        )
                        nc.vector.tensor_scalar(
                            so1[:, f0:f0 + nf], po[0:p_n1, 0:nf],
                            bias1, 0.0, add_op, max_op,
                        )
                    f0 += nf
                # stores all ride the sync HWDGE ring: fast RTL triggers on
                # an otherwise-idle engine, and loads (gpsimd ring) are
                # never stuck behind them
                nc.sync.dma_start(out_bkl[c, b00:b00 + nb0, :], so0[:, :])
                nc.sync.dma_start(out_bkl[c, b01:b01 + nb1, :], so1[:, :])
    nc.finalize()
    return nc


def _host_prep(eos_emb, bin_ids, emb_table, eos_W, eos_b, fc_W, fc_b, cfg=None):
    """Build the per-core input maps."""
    import ml_dtypes

    bf16 = ml_dtypes.bfloat16
    g = _derived(cfg or DEFAULT_CFG)
    B, C, L, D, E, K = g["B"], g["C"], g["L"], g["D"], g["E"], g["K"]
    NCORES, LC, BC, EP, NQ, ROWS = (
        g["NCORES"], g["LC"], g["BC"], g["EP"], g["NQ"], g["ROWS"])

    eos_emb = np.ascontiguousarray(eos_emb, dtype=np.float32)
    emb_table = np.ascontiguousarray(emb_table, dtype=np.float32)
    bin_ids = np.asarray(bin_ids)

    # gather (identity when bin_ids == arange, which is the spec'd fill)
    V = C * L
    flat_ids = bin_ids.reshape(-1)
    if flat_ids.shape[0] == V and emb_table.shape[0] == V and \
            flat_ids[0] == 0 and flat_ids[-1] == V - 1 and \
            np.array_equal(flat_ids, np.arange(V, dtype=flat_ids.dtype)):
        bin_emb = emb_table.reshape(C, L, D)
    else:
        bin_emb = emb_table[bin_ids.reshape(C, L)]

    eosE = np.ascontiguousarray(
        eos_emb.reshape(BC, E).T.reshape(NQ, EP, BC).transpose(1, 0, 2).reshape(EP, NQ * BC)
    ).astype(bf16)
    fc_W = np.asarray(fc_W, np.float32)
    eos_W = np.asarray(eos_W, np.float32)
    # weights-only folds:  W2 = We @ eos_W  [K, E],  bias2 = We@eos_b + fc_b
    We = fc_W[:, D:]                                     # [K, D]
    W2 = We @ eos_W                                      # [K, E]
    W2T = np.ascontiguousarray(
        W2.T.reshape(NQ, EP, K).transpose(1, 0, 2).reshape(EP, NQ * K)
    ).astype(bf16)
    bias2 = (We @ np.asarray(eos_b, np.float32).reshape(D)
             + np.asarray(fc_b, np.float32)).reshape(K, 1).astype(np.float32)
    # wsel[d, c*ROWS + b*K + k] = fc_W[k, d] / 32  (embT carries the x32)
    wsel1 = np.tile(fc_W[:, :D] / 32.0, (B, 1)).T        # [D, B*K] (b-major)
    wsel = np.ascontiguousarray(np.tile(wsel1, (1, C))).astype(bf16)

    shared = dict(eosE=eosE, W2T=W2T, bias2=bias2, wsel=wsel)

    import concourse.mybir as mybir

    fp8 = mybir.dt.np(mybir.dt.float8e4)
    in_maps = []
    for i in range(NCORES):
        sl = bin_emb[:, i * LC:(i + 1) * LC, :]          # [C, Lc, D]
        embT_i = np.ascontiguousarray(
            sl.transpose(2, 0, 1).reshape(D, C * LC) * np.float32(32.0)
        ).astype(fp8)
        in_maps.append({"embT": embT_i, **shared})
    return in_maps


def _assemble(results, cfg=None):
    g = _derived(cfg or DEFAULT_CFG)
    B, C, L, K, NCORES, LC = g["B"], g["C"], g["L"], g["K"], g["NCORES"], g["LC"]
    out = np.empty((B, C, L, K), np.float32)
    for i in range(NCORES):
        r = results[i]["out"]                            # [B, C, K, Lc] bf16
        out[:, :, i * LC:(i + 1) * LC, :] = r.transpose(0, 1, 3, 2).astype(np.float32)
    return out


def kernel(eos_emb, bin_ids, emb_table, eos_W, eos_b, fc_W, fc_b):
    from concourse.bass_utils import run_bass_kernel_spmd

    if "nc" not in _CACHE:
        _CACHE["nc"] = _build_nc()
    nc = _CACHE["nc"]
    in_maps = _host_prep(eos_emb, bin_ids, emb_table, eos_W, eos_b, fc_W, fc_b)
    res = run_bass_kernel_spmd(nc, in_maps, core_ids=list(range(DEFAULT_CFG["NCORES"])))
    return _assemble(res.results)
